# revision 1
# baseline (speedup 1.0000x reference)
"""DynamicFilter Trainium2 kernel.

Computation (per sample b):
    h  = tanh(query @ W1.T + b1)                      [B, 256]
    cw = (h @ W2.T + b2).reshape(B, C=32, K=31)       per-sample conv weights
    x[b,t,c] = sum_k cw[b,c,k] * pad(prev_attn)[b, t+k]
    out[b,t,o] = sum_c Wfc[o,c] x[b,t,c] + bfc[o]

Key algebraic fusion: fold the fc into the conv,
    Weff[b,o,k] = sum_c Wfc[o,c] cw[b,c,k]            [B, 128, 31]
    out[b,t,o]  = sum_k Weff[b,o,k] pad(prev_attn)[b, t+k] + bfc[o]
so the T-sized work is ONE matmul per (sample, 512-wide t-chunk):
    psum[128 o, 512 t] = WeffT_b[32 k, 128 o].T @ windows[32 k, 512 t]
with the windows operand streamed from SBUF tiles holding 31 shifted
replicas of each padded row plus a row of ones; the matching 32nd
stationary row holds bfc, so PSUM accumulates conv + bias exactly in
fp32 and the psum->sbuf drain is a plain dtype-narrowing copy.

The whole T-sized data path runs in bf16 (the correctness gate is
rel_err < 2e-2; bf16 rounding costs ~5e-3): replicas, matmul operands
and the output stream are all bf16, halving HBM traffic -- the f32
profile showed all 16 SDMA engines pegged at the ~350 GB/s HBM
roofline.  PSUM accumulation stays fp32.

Main loop: 64 conv matmuls (8 samples x 8 psum-bank-capped 512-column
chunks) kept gap-free -- 5 PSUM banks round-robin, drain copies
alternate DVE/ACT (the only PSUM-reading engines), out-DMA dispatches
live on the otherwise-idle sync queue, and the next group's Weff prep
is emitted mid-stream (in-order engine queues: emitting it up front
head-of-line-blocks the drains behind the slow later-group gathers).

Head latency tricks (the ~7us framework preamble and ~2.3us HWDGE
dispatch->data latency dominate the hypernet phase): one st0 DMA
carries query + the first w1 chunks; b2/bfc are host-folded
(B2effT = Wfc@b2 added at the Weff drain, bfc rows DMA'd at staging);
each group's per-sample conv weights move in ONE sbuf->sbuf gather
(cwS[32i+c, k] <- cwB[b0+i, 31c+k] -- dst and src both iterate
(i, c, k)), paired with a per-32-partition-replicated WfcT so the weff
matmul operands share base partitions.

Sharding: data-parallel over batch. 64 samples / 8 cores = 8 per core.
Weights replicated. Output written bf16 [b, o, t] in [128, 2048] tiles
(4 KB contiguous runs; the last sample drains in 1024-column tiles to
shorten the tail); host upcasts and returns a transposed view
[B, T, O] in f32.
"""

import sys

import numpy as np

if "/opt/trn_rl_repo" not in sys.path:
    sys.path.insert(0, "/opt/trn_rl_repo")

from contextlib import ExitStack

import ml_dtypes

import concourse.bass as bass
import concourse.mybir as mybir
import concourse.tile as tile
from concourse import bacc
from concourse.ap import AP
from concourse.bass_utils import run_bass_kernel_spmd
from concourse.masks import make_identity

# Problem shapes (hardcoded per contract).
B, T = 64, 4096
D, H = 1024, 256
C, K, O = 32, 31, 128
KB = K + 1  # conv taps + the folded-bias ones row
PAD = (K - 1) // 2  # 15
NCORES = 8
BPC = B // NCORES  # 8 samples per core
TCH = 512  # t-chunk (matmul moving free dim, one PSUM bank)
NT = T // TCH  # 8 chunks per sample
OCH = 2048  # out-tile column width (4 KB bf16 runs)
PCH = 2048  # replica chunk width (4 KB bf16 runs)
GROUPS = [(0, 3), (3, 3), (6, 2)]  # (first sample, count) per replica tile

F32 = mybir.dt.float32
BF16 = mybir.dt.bfloat16
AF = mybir.ActivationFunctionType
BF16NP = ml_dtypes.bfloat16

_CACHED = {}


def _build_nc():
    nc = bacc.Bacc(
        "TRN2", target_bir_lowering=False, debug=False, num_devices=NCORES
    )
    # leave semaphore waits on the matmuls: a wait-free LDWEIGHTS can
    # prefetch into the PE shadow registers while the previous matmul is
    # still streaming (generate_event_semaphores still enforces the
    # 1-wait-per-instruction constraint)
    nc.move_matmul_waits_to_ldweights = lambda: None

    # host-prepacked layouts: single contiguous DMAs into the exact SBUF
    # images (descriptor count on the HWDGE rings is a scarce resource)
    # st0 = [qtp | all 8 w1 d-chunks]: ONE first DMA covers everything
    # mm1 needs -- it starts ~0.7us later than a split would allow but
    # can never stall mid-stream on a late chunk (whole-tile deps make
    # split staging nondeterministic under ring-timing jitter)
    qT_h = nc.dram_tensor("st0", [128, 8 * BPC + 8 * H], BF16,
                          kind="ExternalInput")
    rep_h = nc.dram_tensor("paRep", [len(GROUPS), 96, T], BF16,
                           kind="ExternalInput")
    w1t_h = nc.dram_tensor("w1tp", [128, 8 * H], BF16, kind="ExternalInput")
    b1_h = nc.dram_tensor("b1p", [128, 2], F32, kind="ExternalInput")
    w2t_h = nc.dram_tensor("w2tp", [128, 2 * C * K], BF16,
                           kind="ExternalInput")
    wfct_h = nc.dram_tensor("wfct96", [96, O], BF16, kind="ExternalInput")
    b2t_h = nc.dram_tensor("b2t", [96, O], F32, kind="ExternalInput")
    bfcr_h = nc.dram_tensor("bfcr", [3, O], BF16, kind="ExternalInput")
    out_h = nc.dram_tensor("out", [BPC, O, T], BF16, kind="ExternalOutput")

    with tile.TileContext(nc) as tc:
        _emit(tc, qT_h, rep_h, w1t_h, b1_h, w2t_h, b2t_h, wfct_h, bfcr_h,
              out_h)

    nc.compile()
    return nc


def _emit(tc, qT_h, rep_h, w1t_h, b1_h, w2t_h, b2t_h, wfct_h, bfcr_h, out_h):
    nc = tc.nc
    with ExitStack() as ctx:
        singles = ctx.enter_context(tc.tile_pool(name="singles", bufs=1))
        cw_pool = ctx.enter_context(tc.tile_pool(name="cw", bufs=3))
        weff_pool = ctx.enter_context(tc.tile_pool(name="weff", bufs=3))
        pa_pool = ctx.enter_context(tc.tile_pool(name="pa", bufs=6))
        out_pool = ctx.enter_context(tc.tile_pool(name="outsb", bufs=6))
        # one bank shared by the pre-chain (ph/pt, done by ~15us) and the
        # weff tiles (first used after), so the main loop keeps 5 banks
        psum_pre = ctx.enter_context(
            tc.tile_pool(name="psum_pre", bufs=1, space="PSUM")
        )
        psum_mm2 = ctx.enter_context(
            tc.tile_pool(name="psum_mm2", bufs=2, space="PSUM")
        )
        psum_weff = psum_pre
        psum_main = ctx.enter_context(
            tc.tile_pool(name="psum_main", bufs=5, space="PSUM")
        )

        # ---- staging.  Scalar's queue head carries the ACT table load,
        # so the latency-critical first weight DMAs go on sync/gpsimd;
        # engines round-robin between queue rows at packet granularity.
        rep_ap = rep_h.ap()
        # per-(group, chunk) tiles: finer deps, group-0 matmuls start early
        pa_tiles = [
            [
                pa_pool.tile([96, PCH], BF16, tag="pa", name=f"pa_g{g}c{c}")
                for c in range(T // PCH)
            ]
            for g in range(len(GROUPS))
        ]

        # st0 tile: qt_sb[p, (dc, b)] = qT[128*dc + p, b] plus all 8 w1
        # chunks, first DMA on the sync HW ring
        st0_sb = singles.tile([128, 8 * BPC + 8 * H], BF16)
        nc.sync.dma_start(st0_sb[:], qT_h.ap())
        qt_sb = st0_sb[:, 0 : 8 * BPC]
        w1t_tiles = [
            st0_sb[:, 8 * BPC + H * j : 8 * BPC + H * j + H] for j in range(8)
        ]
        # w2t in two halves; half hc holds both mm2 operands for that
        # contraction chunk (chunk 2*hc + nh = its 496-column slice)
        w2_half = []
        for hc in range(2):
            w2c = singles.tile([128, 2 * 496], BF16, name=f"w2h{hc}")
            eng = nc.sync if hc == 0 else nc.gpsimd
            eng.dma_start(
                w2c[:], w2t_h.ap()[:, 992 * hc : 992 * hc + 992]
            )
            w2_half.append(w2c)
        w2t_tiles = [
            w2_half[ch // 2][:, 496 * (ch % 2) : 496 * (ch % 2) + 496]
            for ch in range(4)
        ]
        # WfcT replicated per 32-partition sample slot: weff matmul rhs
        # slices share their base partition with the stacked-cw lhsT
        wfct_sb = singles.tile([96, O], BF16)
        nc.gpsimd.dma_start(wfct_sb[:], wfct_h.ap())
        # B2effT[32i + k, o] = sum_c Wfc[o,c] b2[31c + k] (host-folded b2)
        b2t_sb = singles.tile([96, O], F32)
        nc.gpsimd.dma_start(b2t_sb[:], b2t_h.ap())
        b1_sb = singles.tile([128, 2], F32)
        nc.scalar.dma_start(b1_sb[:], b1_h.ap())
        # replica groups in 2KB-run column chunks on the sync ring
        for gi, (b0, cnt) in enumerate(GROUPS):
            for ch in range(T // PCH):
                nc.sync.dma_start(
                    pa_tiles[gi][ch][0 : 32 * cnt, :],
                    rep_ap[gi, 0 : 32 * cnt, PCH * ch : PCH * ch + PCH],
                )
        ident_sb = singles.tile([BPC, BPC], F32)
        make_identity(nc, ident_sb[:])

        # weff tiles pre-created so their bfc rows {31, 63, 95} can be
        # DMA'd at staging time (no data deps; pairs with the replica
        # ones row).  gpsimd's queue is idle after its staging share.
        weff_tiles = []
        for gi, (b0, cnt) in enumerate(GROUPS):
            wg = weff_pool.tile([96, O], BF16, tag="weff", name=f"wg{gi}")
            for i in range(cnt):
                nc.gpsimd.dma_start(
                    wg[32 * i + K : 32 * i + KB, :], bfcr_h.ap()[i : i + 1, :]
                )
            weff_tiles.append(wg)

        # ---- hypernet mm1 (wide-N orientation): h[b, j] --------------
        # h[b, j] = sum_d qT[d, b] W1T[d, j]
        ph = psum_pre.tile([BPC, H], F32, tag="pre")
        for dc in range(8):
            nc.tensor.matmul(
                ph[:],
                lhsT=qt_sb[:, BPC * dc : BPC * dc + BPC],
                rhs=w1t_tiles[dc],
                start=(dc == 0),
                stop=(dc == 7),
            )
        h_sb = singles.tile([BPC, H], F32)
        # b1 is applied with tanh after the transpose (bias varies along
        # the free dim in this layout), so copy raw here -- in two halves
        # on two engines, so transpose jc waits only on its own half
        nc.vector.tensor_copy(h_sb[:, 0:128], ph[:, 0:128])
        nc.scalar.activation(h_sb[:, 128:256], ph[:, 128:256], AF.Identity)

        # transpose h -> hT chunks [128 j, BPC] and apply tanh(+b1) there;
        # the two transposes use the 2-bank mm2 pool so they overlap
        htr_sb = singles.tile([128, 2 * BPC], BF16)
        for jc in range(2):
            pt = psum_mm2.tile([128, BPC], F32, tag="mm2")
            nc.tensor.transpose(
                pt[:], h_sb[:, 128 * jc : 128 * jc + 128], ident_sb[:]
            )
            nc.scalar.activation(
                htr_sb[:, BPC * jc : BPC * jc + BPC], pt[:], AF.Tanh,
                bias=b1_sb[:, jc : jc + 1],
            )

        # ---- hypernet mm2: cwB[b, (c k)] = sum_h W2T[h, ck] hT[h, b] ----
        # (b2 is host-folded into B2effT and added at the Weff copy)
        # mm2's cost is column-count only (samples sit on psum partition
        # rows), so group 0's 3 samples are computed FIRST at full cost
        # parity: their gather dispatches ~2.5us earlier, and the
        # remaining samples' mm2 fills the tensor pipe during the
        # gather's ~2.5us HWDGE ring latency.
        HALF = C * K // 2  # 496
        # two tiles so each split's psum drain lands at partition base 0
        # (engine copies can't shift partitions; DMAs can)
        cwB0_sb = singles.tile([3, C * K], BF16)
        cwB1_sb = singles.tile([5, C * K], BF16)
        cwB_parts = [(cwB0_sb, 0), (cwB1_sb, 0), (cwB1_sb, 3)]  # per group
        cws_tiles = [
            cw_pool.tile([96, K], BF16, tag="cws", name=f"cws{gi}")
            for gi in range(len(GROUPS))
        ]

        def emit_gather(gi):
            # per-group stacked cw gather, ONE dispatch per group:
            #   cwS[32 i + c, k] <- cwB[b0 + i, 31 c + k]
            # dst iterates (i, c, k) partition-major and src iterates the
            # same (partition b, then free dims reordered c-major), so
            # the whole group moves in a single sbuf->sbuf DMA of
            # contiguous 62-byte runs.  Group 0 on the scalar HWDGE ring
            # (hardware descgen + idle here; sync is still streaming
            # replicas); later groups aren't needed for microseconds and
            # take the gpsimd path.
            b0, cnt = GROUPS[gi]
            src_tile, row = cwB_parts[gi]
            eng = nc.scalar if gi == 0 else nc.gpsimd
            eng.dma_start(
                cws_tiles[gi][0 : 32 * cnt, :],
                src_tile[row : row + cnt, :].rearrange(
                    "p (c k) -> p c k", c=C
                ),
            )

        for si, (r0, nr, cwb) in enumerate([(0, 3, cwB0_sb), (3, 5, cwB1_sb)]):
            for nh in range(2):
                pc = psum_mm2.tile([nr, HALF], F32, tag="mm2",
                                   name=f"pc{si}_{nh}")
                for hc in range(2):
                    nc.tensor.matmul(
                        pc[:],
                        lhsT=htr_sb[:, BPC * hc + r0 : BPC * hc + r0 + nr],
                        rhs=w2t_tiles[2 * hc + nh],
                        start=(hc == 0),
                        stop=(hc == 1),
                    )
                dst = cwb[:, HALF * nh : HALF * nh + HALF]
                if nh == 0:
                    nc.vector.tensor_copy(dst, pc[:])
                else:
                    nc.scalar.activation(dst, pc[:], AF.Identity)
            if si == 0:
                emit_gather(0)
            else:
                emit_gather(1)
                emit_gather(2)

        pw_tiles = [None] * len(GROUPS)

        def emit_weff_mm(gi):
            # WeffT_b[k, o] = sum_c cw_b[c, k] WfcT[c, o]; sample i of a
            # group lives at partition base 32*i in both operands
            b0, cnt = GROUPS[gi]
            pw = psum_weff.tile([96, O], F32, tag="pre", name=f"pw{gi}")
            pw_tiles[gi] = pw
            for i in range(cnt):
                nc.tensor.matmul(
                    pw[32 * i : 32 * i + K, :],
                    lhsT=cws_tiles[gi][32 * i : 32 * i + C, :],
                    rhs=wfct_sb[32 * i : 32 * i + C, :],
                    start=True,
                    stop=True,
                )

        def emit_stt(gi, i):
            # wg = pw + B2effT (the host-folded b2 contribution), emitted
            # per sample right before its chunks: sample i's first matmul
            # then waits on ONE copy, not the whole group's, and the DVE
            # load spreads instead of stalling the psum drains
            nc.vector.scalar_tensor_tensor(
                weff_tiles[gi][32 * i : 32 * i + K, :],
                pw_tiles[gi][32 * i : 32 * i + K, :],
                1.0,
                b2t_sb[32 * i : 32 * i + K, :],
                mybir.AluOpType.mult,
                mybir.AluOpType.add,
            )

        # ---- main loop: keep the PE stream gap-free.  Group gi+1's weff
        # work is emitted mid-way through group gi's samples: the engine
        # queues are in-order, so emitting it up front would head-of-line
        # block the main-loop drains behind the slow later-group gathers.
        emit_weff_mm(0)
        idx = 0
        out_ap = out_h.ap()
        for gi, (b0, cnt) in enumerate(GROUPS):
            wg = weff_tiles[gi]
            for i in range(cnt):
                emit_stt(gi, i)
                if i == cnt - 1 and gi + 1 < len(GROUPS):
                    emit_weff_mm(gi + 1)
                lhsT = wg[32 * i : 32 * i + KB, :]
                b = b0 + i
                # finer out tiles for the final sample: the last DMA
                # starts two psum drains earlier, shortening the tail
                och = OCH // 2 if b == BPC - 1 else OCH
                for oc in range(T // och):
                    osb = out_pool.tile([O, och], BF16, tag="osb",
                                        name=f"osb{b}_{oc}")
                    for q in range(och // TCH):
                        tcn = oc * (och // TCH) + q
                        pm = psum_main.tile([O, TCH], F32, tag="pmm")
                        nc.tensor.matmul(
                            pm[:],
                            lhsT=lhsT,
                            rhs=pa_tiles[gi][tcn // (PCH // TCH)][
                                32 * i : 32 * i + KB,
                                TCH * (tcn % (PCH // TCH)) :
                                TCH * (tcn % (PCH // TCH)) + TCH,
                            ],
                            start=True,
                            stop=True,
                        )
                        # psum -> sbuf bf16 narrowing copy (bias already in);
                        # only DVE and ACT can read PSUM -- alternate them
                        dst = osb[:, TCH * q : TCH * q + TCH]
                        if idx % 2 == 0:
                            nc.vector.tensor_copy(dst, pm[:])
                        else:
                            nc.scalar.activation(dst, pm[:], AF.Identity)
                        idx += 1
                    # the last sample's tiles go out on the scalar HW
                    # ring: sync's ring still has a packet backlog from
                    # earlier tiles, scalar's is empty by now
                    out_eng = nc.scalar if b == BPC - 1 else nc.sync
                    out_eng.dma_start(
                        out_ap[b, :, och * oc : och * oc + och], osb[:]
                    )


def get_nc(use_f32r=True):
    # use_f32r kept for test-harness compat; the data path is bf16.
    if "nc" not in _CACHED:
        _CACHED["nc"] = _build_nc()
    return _CACHED["nc"]


def make_in_maps(query, prev_attn, W1, b1, W2, b2, Wfc, bfc):
    """Shard + lay out host inputs for the 8 cores."""
    f = np.float32
    w1t = np.asarray(W1, f).T  # [D, H]
    w2t = np.asarray(W2, f).T  # [H, C*K]
    # WfcT replicated per 32-partition sample slot, bf16
    wfct96 = np.zeros((96, O), BF16NP)
    for i in range(3):
        wfct96[32 * i : 32 * i + C] = np.asarray(Wfc, f).T.astype(BF16NP)
    b1 = np.asarray(b1, f)
    # host-folded b2: B2effT[k, o] = sum_c Wfc[o, c] b2[31c + k],
    # replicated per 32-partition sample slot (row 32i+31 unused -> 0)
    b2eff = np.asarray(Wfc, f) @ np.asarray(b2, f).reshape(C, K)  # [O, K]
    b2t = np.zeros((96, O), f)
    for i in range(3):
        b2t[32 * i : 32 * i + K] = b2eff.T
    bfcr = np.ascontiguousarray(
        np.broadcast_to(np.asarray(bfc, f).reshape(1, O), (3, O))
    ).astype(BF16NP)
    query = np.asarray(query, f)
    prev_attn = np.asarray(prev_attn, f)

    # prepack into the SBUF partition-major images the kernel DMAs verbatim
    # w1tp[p, (dc, j)] = W1T[128*dc + p, j]
    w1tp = np.ascontiguousarray(
        w1t.reshape(8, 128, H).transpose(1, 0, 2).reshape(128, 8 * H)
    ).astype(BF16NP)
    w2tp = np.ascontiguousarray(
        w2t.reshape(2, 128, C * K).transpose(1, 0, 2).reshape(128, 2 * C * K)
    ).astype(BF16NP)
    b1p = np.ascontiguousarray(b1.reshape(2, 128).T)  # [128, 2]

    in_maps = []
    for i in range(NCORES):
        sl = slice(i * BPC, (i + 1) * BPC)
        qT = query[sl].T  # [D, BPC]
        qtp = np.ascontiguousarray(
            qT.reshape(8, 128, BPC).transpose(1, 0, 2).reshape(128, 8 * BPC)
        ).astype(BF16NP)
        st0 = np.ascontiguousarray(np.concatenate([qtp, w1tp], axis=1))
        # shifted replicas: paRep[g, 32*i + k, t] = pad(prev_attn)[b0+i, k+t]
        # with row 32*i + 31 = ones (pairs with the bfc row in Weff)
        padded = np.zeros((BPC, T + 2 * PAD), f)
        padded[:, PAD : PAD + T] = prev_attn[sl]
        win = np.lib.stride_tricks.sliding_window_view(padded, T, axis=1)
        # win[b, k, t] = padded[b, k + t], k in [0, 31)
        rep = np.zeros((len(GROUPS), 96, T), BF16NP)
        for g, (b0, cnt) in enumerate(GROUPS):
            for j in range(cnt):
                rep[g, 32 * j : 32 * j + K] = win[b0 + j].astype(BF16NP)
                rep[g, 32 * j + K] = BF16NP(1.0)
        in_maps.append(
            {
                "st0": st0,
                "paRep": rep,
                "w1tp": w1tp,
                "b1p": b1p,
                "w2tp": w2tp,
                "b2t": b2t,
                "wfct96": wfct96,
                "bfcr": bfcr,
            }
        )
    return in_maps


def assemble_output(results):
    """[8 cores] x [BPC, O, T] bf16 -> [B, T, O] f32 view."""
    full = np.concatenate(
        [r["out"].astype(np.float32) for r in results], axis=0
    )  # [B, O, T]
    return full.transpose(0, 2, 1)


def kernel(query, prev_attn, W1, b1, W2, b2, Wfc, bfc):
    nc = get_nc()
    in_maps = make_in_maps(query, prev_attn, W1, b1, W2, b2, Wfc, bfc)
    res = run_bass_kernel_spmd(nc, in_maps, list(range(NCORES)))
    return assemble_output(res.results)



# revision 14
# speedup vs baseline: 1.2291x; 1.2291x over previous
"""DynamicFilter Trainium2 kernel.

Computation (per sample b):
    h  = tanh(query @ W1.T + b1)                      [B, 256]
    cw = (h @ W2.T + b2).reshape(B, C=32, K=31)       per-sample conv weights
    x[b,t,c] = sum_k cw[b,c,k] * pad(prev_attn)[b, t+k]
    out[b,t,o] = sum_c Wfc[o,c] x[b,t,c] + bfc[o]

Key algebraic fusion: fold the fc into the conv,
    Weff[b,o,s] = sum_c Wfc[o,c] cw[b,c,s]            [B, 128, 31]
    out[b,t,o]  = sum_s Weff[b,o,s] pad(prev_attn)[b, t+s] + bfc[o]
so the T-sized work is ONE matmul per (sample, 512-wide t-chunk):
    psum[128 o, 512 t] = WeffT_b[32 s, 128 o].T @ windows[32 s, 512 t]
with the windows operand streamed from SBUF tiles holding 31 shifted
replicas of each padded row plus a row of ones; the matching 32nd
stationary row holds bfc, so PSUM accumulates conv + bias exactly in
fp32 and the psum->sbuf drain is a plain dtype-narrowing copy.

The whole T-sized data path runs in bf16 (correctness gate is
rel_err < 2e-2; bf16 rounding costs ~5e-3). PSUM accumulation is fp32.

Head latency is the battle: the framework preamble + first-DMA
latency means nothing computes before ~8.5us, so the hypernet ->
conv-weight chain must be short and DMA-free.  The v0 kernel's
per-group cw gather (sbuf->sbuf DMA) cost ~8us of HWDGE round-trip
latency on the critical path; this version never lets the conv-weight
data leave the engines:

  mm1' : hT[j, b] computed directly j-on-partitions (16 small
         matmuls, lhsT = host-prepacked W1 slices), so tanh(+b1) is a
         single ACT from PSUM with a per-partition bias -- no PE
         transpose, no identity matrix.
  mm2  : W2 columns host-reordered to (slot, channel): col = 32 s + c
         (slot 31 zero-padded), so cwB[8 b, 1024] comes out in 4 wide
         matmuls.
  DVE StreamTranspose (32x32 blocks, sbuf->sbuf, no DMA):
         cwB[32, 1024] -> cwT[32, 1024] with cwT[c, 32 s + b]
         = cw[b, c, s] -- the per-sample weight matrices land
         c-on-partitions in one instruction.
  weff : 2 matmuls, one per 4-sample quad: lhsT = cwT cols
         (i: stride 1, s: stride 32) -> psum[32 i + s, o]; rhs = WfcT.
         The stt drain adds host-folded B2eff (rows 32i+s) whose row
         31 is bfc (pairs with the replica ones-row).

Main loop: 64 conv matmuls (8 samples x 8 512-column chunks),
round-robin over 6 PSUM banks, drains alternating DVE/ACT (the only
PSUM-reading engines), out-DMA on the sync HWDGE ring.  The final
sample's tiles are finer (1024 cols) and alternate scalar/sync rings
to shorten the tail.

Sharding: data-parallel over batch. 64 samples / 8 cores = 8 per core.
Weights replicated. Output written bf16 [b, o, t]; host upcasts and
returns a transposed view [B, T, O] in f32.
"""

import sys

import numpy as np

if "/opt/trn_rl_repo" not in sys.path:
    sys.path.insert(0, "/opt/trn_rl_repo")

from contextlib import ExitStack

import ml_dtypes

import concourse.bass as bass
import concourse.mybir as mybir
import concourse.tile as tile
from concourse import bacc
from concourse.ap import AP
from concourse.bass_utils import run_bass_kernel_spmd

# Problem shapes (hardcoded per contract).
B, T = 64, 4096
D, H = 1024, 256
C, K, O = 32, 31, 128
S = 32  # slot count: 31 conv taps + the folded-bias ones row
PAD = (K - 1) // 2  # 15
NCORES = 8
BPC = B // NCORES  # 8 samples per core
TCH = 512  # t-chunk (matmul moving free dim, one PSUM bank)
NT = T // TCH  # 8 chunks per sample
OCH = 2048  # out-tile column width (4 KB bf16 runs)
PCH = 2048  # replica chunk width (4 KB bf16 runs)
# PE row/psum quadrant bases only allow {0, 32, 64}, so samples group in
# threes (3 + 3 + 2) for the weff/conv stages
GROUPS = [(0, 3), (3, 3), (6, 2)]

F32 = mybir.dt.float32
BF16 = mybir.dt.bfloat16
AF = mybir.ActivationFunctionType
BF16NP = ml_dtypes.bfloat16

_CACHED = {}


def _build_nc():
    nc = bacc.Bacc(
        "TRN2", target_bir_lowering=False, debug=False, num_devices=NCORES
    )
    # leave semaphore waits on the matmuls: a wait-free LDWEIGHTS can
    # prefetch into the PE shadow registers while the previous matmul is
    # still streaming (generate_event_semaphores still enforces the
    # 1-wait-per-instruction constraint)
    nc.move_matmul_waits_to_ldweights = lambda: None

    # host-prepacked layouts: contiguous DMAs into the exact SBUF images
    # st0a = [qtp | W1 chunk dc=0]; st0b = W1 chunks dc=1..7 -- split so
    # mm1 starts on chunk 0 while the rest streams in behind it
    st0a_h = nc.dram_tensor("st0a", [128, BPC * 8 + 2 * 128], BF16,
                            kind="ExternalInput")
    st0b_h = nc.dram_tensor("st0b", [128, 14 * 128], BF16,
                            kind="ExternalInput")
    w2s_h = nc.dram_tensor("w2s", [128, 2 * S * C], BF16,
                           kind="ExternalInput")
    b1_h = nc.dram_tensor("b1p", [128, 2], F32, kind="ExternalInput")
    wfct_h = nc.dram_tensor("wfct", [C, O], BF16, kind="ExternalInput")
    b2t_h = nc.dram_tensor("b2t", [96, O], F32, kind="ExternalInput")
    rep_h = nc.dram_tensor("paRep", [len(GROUPS), 96, T], BF16,
                           kind="ExternalInput")
    out_h = nc.dram_tensor("out", [BPC, O, T], BF16, kind="ExternalOutput")

    with tile.TileContext(nc) as tc:
        _emit(tc, st0a_h, st0b_h, w2s_h, b1_h, wfct_h, b2t_h, rep_h, out_h)

    nc.compile()
    return nc


def _emit(tc, st0a_h, st0b_h, w2s_h, b1_h, wfct_h, b2t_h, rep_h, out_h):
    nc = tc.nc
    with ExitStack() as ctx:
        singles = ctx.enter_context(tc.tile_pool(name="singles", bufs=1))
        wg_pool = ctx.enter_context(tc.tile_pool(name="wg", bufs=3))
        pa_pool = ctx.enter_context(tc.tile_pool(name="pa", bufs=6))
        out_pool = ctx.enter_context(tc.tile_pool(name="outsb", bufs=6))
        # hypernet psum: ph -> cw halves -> weff quads rotate in 2 banks
        psum_pre = ctx.enter_context(
            tc.tile_pool(name="psum_pre", bufs=2, space="PSUM")
        )
        psum_main = ctx.enter_context(
            tc.tile_pool(name="psum_main", bufs=6, space="PSUM")
        )

        # ---- staging.  sync ring: st0a, st0b, replicas (in need order);
        # scalar ring: w2s, b1; gpsimd ring: wfct, b2t.
        st0a_sb = singles.tile([128, BPC * 8 + 2 * 128], BF16)
        nc.sync.dma_start(st0a_sb[:], st0a_h.ap())
        qt_sb = st0a_sb[:, 0 : BPC * 8]
        st0b_sb = singles.tile([128, 14 * 128], BF16)
        nc.sync.dma_start(st0b_sb[:], st0b_h.ap())

        def w1sl(dc, jc):
            if dc == 0:
                return st0a_sb[:, BPC * 8 + 128 * jc : BPC * 8 + 128 * jc + 128]
            return st0b_sb[:, 256 * (dc - 1) + 128 * jc :
                           256 * (dc - 1) + 128 * jc + 128]

        w2s_sb = singles.tile([128, 2 * S * C], BF16)
        nc.scalar.dma_start(w2s_sb[:], w2s_h.ap())
        b1_sb = singles.tile([128, 2], F32)
        nc.scalar.dma_start(b1_sb[:], b1_h.ap())
        wfct_sb = singles.tile([C, O], BF16)
        nc.gpsimd.dma_start(wfct_sb[:], wfct_h.ap())
        b2t_sb = singles.tile([96, O], F32)
        nc.gpsimd.dma_start(b2t_sb[:], b2t_h.ap())

        rep_ap = rep_h.ap()
        pa_tiles = [
            [
                pa_pool.tile([96, PCH], BF16, tag="pa", name=f"pa_g{g}c{c}")
                for c in range(T // PCH)
            ]
            for g in range(len(GROUPS))
        ]
        for g, (b0, cnt) in enumerate(GROUPS):
            for ch in range(T // PCH):
                nc.sync.dma_start(
                    pa_tiles[g][ch][0 : S * cnt, :],
                    rep_ap[g, 0 : S * cnt, PCH * ch : PCH * ch + PCH],
                )

        # cwB rows 8..31 are never written by mm2's drain but ARE read by
        # the stream transpose (their transposed columns are unused) --
        # memset them so the sim never sees uninitialized reads
        cwB_sb = singles.tile([S, S * C], BF16)
        cwT_sb = singles.tile([S, S * C], BF16)
        # (whole tile: engine ops need 32-aligned start partitions; the
        # mm2 drains overwrite rows 0..7 afterwards)
        nc.gpsimd.memset(cwB_sb[:], 0.0)

        # ---- hypernet mm1' (transposed orientation): hT[j, b] ----------
        # hT[128 jc + jj, b] = sum_d W1[j, d] qT[d, b]; lhsT = W1 slices.
        # Two accumulation chains interleave dc-major (so chunk-0 work
        # starts before st0b lands); separate psum tiles keep their
        # zero regions apart.
        ph = [
            psum_pre.tile([128, BPC], F32, tag="pre", name=f"ph{jc}")
            for jc in range(2)
        ]
        for dc in range(8):
            for jc in range(2):
                nc.tensor.matmul(
                    ph[jc][:],
                    lhsT=w1sl(dc, jc),
                    rhs=qt_sb[:, BPC * dc : BPC * dc + BPC],
                    start=(dc == 0),
                    stop=(dc == 7),
                )
        htr_sb = singles.tile([128, 2 * BPC], BF16)
        for jc in range(2):
            nc.scalar.activation(
                htr_sb[:, BPC * jc : BPC * jc + BPC],
                ph[jc][:],
                AF.Tanh,
                bias=b1_sb[:, jc : jc + 1],
            )

        # ---- hypernet mm2: cwB[b, 32 s + c] = sum_h hT[h, b] W2s[h, sc]
        # (w2s columns host-reordered (slot, channel), slot 31 zeroed;
        # b2 is host-folded into B2eff and added at the stt drain)
        HW = S * C // 2  # 512
        cwp = []
        for half in range(2):
            pc = psum_pre.tile([BPC, HW], F32, tag="pre", name=f"cw{half}")
            cwp.append(pc)
            for hc in range(2):
                nc.tensor.matmul(
                    pc[:],
                    lhsT=htr_sb[:, BPC * hc : BPC * hc + BPC],
                    rhs=w2s_sb[:, S * C * hc + HW * half :
                               S * C * hc + HW * half + HW],
                    start=(hc == 0),
                    stop=(hc == 1),
                )
        # drain halves on the two PSUM-reading engines in parallel
        nc.vector.tensor_copy(cwB_sb[0:BPC, 0:HW], cwp[0][:])
        nc.scalar.activation(cwB_sb[0:BPC, HW : 2 * HW], cwp[1][:],
                             AF.Identity)

        # ---- 32x32 block transpose on DVE: cwT[c, 32 s + b] = cw[b,c,s]
        nc.vector.transpose(cwT_sb[:], cwB_sb[:])

        # ---- weff: one matmul per sample (the stationary AP must be a
        # single free dim: cwT cols {32 s + b} at stride 32) ------------
        # psum[32 i + s, o] = sum_c cwT[c, 32 s + (b0 + i)] WfcT[c, o]
        cwT_bs = cwT_sb[:].rearrange("p (s b) -> p b s", s=S)
        wg_tiles = []
        for g, (b0, cnt) in enumerate(GROUPS):
            pw = psum_pre.tile([S * cnt, O], F32, tag="pre", name=f"pw{g}")
            for i in range(cnt):
                nc.tensor.matmul(
                    pw[S * i : S * i + S, :],
                    lhsT=cwT_bs[:, b0 + i, :],
                    rhs=wfct_sb[:],
                    start=True,
                    stop=True,
                )
            wg = wg_pool.tile([S * cnt, O], BF16, tag="weff", name=f"wg{g}")
            # wg = pw + B2eff; B2eff row 32i+31 = bfc (pairs with the
            # replica ones-row), rows 32i+s = host-folded Wfc @ b2
            nc.vector.scalar_tensor_tensor(
                wg[:], pw[:], 1.0, b2t_sb[0 : S * cnt, :],
                mybir.AluOpType.mult, mybir.AluOpType.add,
            )
            wg_tiles.append(wg)

        # ---- main loop: keep the PE stream gap-free --------------------
        idx = 0
        out_ap = out_h.ap()
        for g, (b0, cnt) in enumerate(GROUPS):
            for i in range(cnt):
                b = b0 + i
                lhsT = wg_tiles[g][S * i : S * i + S, :]
                # finer out tiles for the final sample: the last DMAs
                # start earlier and split across two rings
                och = OCH // 2 if b == BPC - 1 else OCH
                for oc in range(T // och):
                    osb = out_pool.tile([O, och], BF16, tag="osb",
                                        name=f"osb{b}_{oc}")
                    for sub in range(och // TCH):
                        tcn = oc * (och // TCH) + sub
                        pm = psum_main.tile([O, TCH], F32, tag="pmm")
                        nc.tensor.matmul(
                            pm[:],
                            lhsT=lhsT,
                            rhs=pa_tiles[g][tcn // (PCH // TCH)][
                                S * i : S * i + S,
                                TCH * (tcn % (PCH // TCH)) :
                                TCH * (tcn % (PCH // TCH)) + TCH,
                            ],
                            start=True,
                            stop=True,
                        )
                        # psum -> sbuf bf16 narrowing copy (bias already
                        # in); only DVE and ACT can read PSUM
                        dst = osb[:, TCH * sub : TCH * sub + TCH]
                        if idx % 2 == 0:
                            nc.vector.tensor_copy(dst, pm[:])
                        else:
                            nc.scalar.activation(dst, pm[:], AF.Identity)
                        idx += 1
                    # the last sample's tiles alternate scalar/sync rings
                    # (scalar's ring is empty after staging)
                    if b == BPC - 1:
                        out_eng = nc.scalar if oc % 2 == 0 else nc.sync
                    else:
                        out_eng = nc.sync
                    out_eng.dma_start(
                        out_ap[b, :, och * oc : och * oc + och], osb[:]
                    )


def get_nc(use_f32r=True):
    # use_f32r kept for test-harness compat; the data path is bf16.
    if "nc" not in _CACHED:
        _CACHED["nc"] = _build_nc()
    return _CACHED["nc"]


def make_in_maps(query, prev_attn, W1, b1, W2, b2, Wfc, bfc):
    """Shard + lay out host inputs for the 8 cores."""
    f = np.float32
    query = np.asarray(query, f)
    prev_attn = np.asarray(prev_attn, f)
    W1 = np.asarray(W1, f)  # [H, D]
    W2 = np.asarray(W2, f)  # [C*K, H]
    Wfc = np.asarray(Wfc, f)  # [O, C]
    b1 = np.asarray(b1, f)
    b2 = np.asarray(b2, f)
    bfc = np.asarray(bfc, f)

    # w1p[p, (dc, jc, jj)] = W1[128 jc + jj, 128 dc + p]
    w1p = np.ascontiguousarray(
        W1.reshape(2, 128, 8, 128).transpose(3, 2, 0, 1).reshape(128, 16 * 128)
    ).astype(BF16NP)
    # w2s[p, (hc, s, c)] = W2[31 c + s, 128 hc + p] for s < 31, else 0
    w2r = W2.reshape(C, K, H)  # [c, k, h]
    w2s = np.zeros((128, 2 * S * C), f)
    for hc in range(2):
        blk = w2r[:, :, 128 * hc : 128 * hc + 128]  # [c, k, p]
        # dst col within half: 32*s + c
        dst = w2s[:, S * C * hc : S * C * (hc + 1)].reshape(128, S, C)
        dst[:, :K, :] = blk.transpose(2, 1, 0)  # [p, k, c]
    w2s = np.ascontiguousarray(w2s).astype(BF16NP)
    b1p = np.ascontiguousarray(b1.reshape(2, 128).T)  # [128, 2]
    wfct = np.ascontiguousarray(Wfc.T).astype(BF16NP)  # [C, O]
    # b2t[32 i + s, o]: s < 31 -> (Wfc @ b2.reshape(C, K))[o, s]; s == 31
    # -> bfc[o]
    b2eff = Wfc @ b2.reshape(C, K)  # [O, K]
    b2t = np.zeros((96, O), f)
    for i in range(3):
        b2t[S * i : S * i + K] = b2eff.T
        b2t[S * i + K] = bfc
    b2t = np.ascontiguousarray(b2t)

    in_maps = []
    for core in range(NCORES):
        sl = slice(core * BPC, (core + 1) * BPC)
        qT = query[sl].T  # [D, BPC]
        qtp = np.ascontiguousarray(
            qT.reshape(8, 128, BPC).transpose(1, 0, 2).reshape(128, 8 * BPC)
        ).astype(BF16NP)
        st0a = np.ascontiguousarray(
            np.concatenate([qtp, w1p[:, 0:256]], axis=1)
        )
        st0b = np.ascontiguousarray(w1p[:, 256:])
        # shifted replicas: paRep[g, 32 i + s, t] = pad(prev_attn)[b, s+t]
        # with row 32 i + 31 = ones (pairs with the bfc row in Weff)
        padded = np.zeros((BPC, T + 2 * PAD), f)
        padded[:, PAD : PAD + T] = prev_attn[sl]
        win = np.lib.stride_tricks.sliding_window_view(padded, T, axis=1)
        # win[b, s, t] = padded[b, s + t], s in [0, 31)
        rep = np.zeros((len(GROUPS), 96, T), BF16NP)
        for g, (b0, cnt) in enumerate(GROUPS):
            for i in range(cnt):
                rep[g, S * i : S * i + K] = win[b0 + i].astype(BF16NP)
                rep[g, S * i + K] = BF16NP(1.0)
        in_maps.append(
            {
                "st0a": st0a,
                "st0b": st0b,
                "w2s": w2s,
                "b1p": b1p,
                "wfct": wfct,
                "b2t": b2t,
                "paRep": rep,
            }
        )
    return in_maps


def assemble_output(results):
    """[8 cores] x [BPC, O, T] bf16 -> [B, T, O] f32 view."""
    full = np.concatenate(
        [r["out"].astype(np.float32) for r in results], axis=0
    )  # [B, O, T]
    return full.transpose(0, 2, 1)


def kernel(query, prev_attn, W1, b1, W2, b2, Wfc, bfc):
    nc = get_nc()
    in_maps = make_in_maps(query, prev_attn, W1, b1, W2, b2, Wfc, bfc)
    res = run_bass_kernel_spmd(nc, in_maps, list(range(NCORES)))
    return assemble_output(res.results)
